# revision 20
# baseline (speedup 1.0000x reference)
"""BranchLinear (MoE routing) Trainium2 kernel.

Math: out[t] = x[t] @ weight[branch_idx[t]] + bias[branch_idx[t]]
  x: [131072, 512] f32, branch_idx: [131072] int32 in [0,8),
  weight: [8, 512, 512] f32, bias: [8, 512] f32.

Strategy (data-parallel over 8 NeuronCores, T sharded, 16384 tokens/core):
  Tokens are processed grouped by branch (host-side argsort of the given
  routing; all FLOPs + HBM traffic on device). Per 2048-token chunk:
    - ONE vectorized `dma_gather(transpose=True)` pulls the sorted token
      rows of bf16 x from HBM and writes them TRANSPOSED into SBUF in
      matmul-stationary layout [128 d, 4 k-chunks, 2048 tok]. This
      replaces the per-128-row indirect DMAs (whose serialized Q7
      descriptor generation dominated the old kernel) and also eliminates
      the PE transpose + PSUM->SBUF copy per tile.
    - per 128-token tile: 4 accumulated bf16 matmuls against the resident
      branch weight (PSUM f32), then DVE adds the pre-broadcast bias.
    - ONE vectorized `dma_scatter_add` writes the f32 rows back to their
      original token slots (output buffer is zero-initialized by the
      runtime; every real row is written exactly once).
  Branch segments are padded to 128-token tiles (pads gather row 0 and
  scatter into trash rows TS..TS+127), sized as the max over cores so one
  SPMD program serves all 8 cores. x/weight are cast to bf16 host-side
  (matmul absmax rel err ~2e-3, well under the 2e-2 gate); bias and
  output stay f32.
"""

import numpy as np
import ml_dtypes

P = 128           # SBUF partitions / tile height (tokens per tile)
NCORES = 8
TPC = 16          # tiles per gather/scatter chunk (2048 tokens)

_prog_cache = {}


def _split_multiwaits(nc):
    """This container's walrus build allows at most ONE sync wait per
    instruction (2 for EventSemaphore), but Tile emits instructions with
    several waits. Hoist extra waits onto fresh single-wait nops inserted
    just before the instruction on the same engine (identical blocking
    semantics: the engine's sequencer executes both in program order)."""
    import concourse.mybir as mybir

    uid = 0
    for f in nc.m.functions:
        for bb in f.blocks:
            insts = bb.instructions
            out, changed = [], False
            for ins in insts:
                si = ins.sync_info
                cap = 2 if ins.opcode == "EventSemaphore" else 1
                if si is not None and len(si.on_wait) > cap:
                    waits = list(si.on_wait)
                    for w in waits[cap:]:
                        nop = mybir.InstNoOp(
                            name=f"waitsplit_{uid}",
                            engine=ins.engine,
                            bass_nofuse=True,
                            sync_info=mybir.SyncInfo(on_wait=[w], on_update=[]),
                        )
                        uid += 1
                        nc.register_instruction(nop, overwrite=True)
                        out.append(nop)
                    si.on_wait = waits[:cap]
                    ins.sync_info = si
                    changed = True
                out.append(ins)
            if changed:
                bb.instructions = out


def _build_program(TS, D, NB, branch_of_tile, epochs=1):
    """Per-core SPMD bass program.

    Inputs (per core): x16 [TS, D] bf16, w16 [NB*D, D] bf16 (weight
    reshaped), br [1, NB*D] f32 (bias), gidx/sidx [128, S*8] int16
    (Ant-wrapped sorted token ids: linear slot n at [n%16, n//16],
    replicated across the 8 Q7 core windows). Output: out [TS+128, D] f32
    (last 128 rows are trash for pad slots; runtime zero-fills,
    scatter-add writes each real row once)."""
    import concourse.bass as bass
    import concourse.mybir as mybir
    import concourse.tile as tile
    from concourse.library_config import mlp

    f32 = mybir.dt.float32
    bf16 = mybir.dt.bfloat16
    i16 = mybir.dt.int16
    KC = D // P                       # contraction chunks (4)
    S = len(branch_of_tile)           # total 128-token tiles

    nc = bass.Bass(name="branch_linear2")
    x_d = nc.dram_tensor("x16", [TS, D], bf16, kind="ExternalInput")
    w_d = nc.dram_tensor("w16", [NB * D, D], bf16, kind="ExternalInput")
    b_d = nc.dram_tensor("br", [1, NB * D], f32, kind="ExternalInput")
    gi_d = nc.dram_tensor("gidx", [128, S * P // 16], i16, kind="ExternalInput")
    si_d = nc.dram_tensor("sidx", [128, S * P // 16], i16, kind="ExternalInput")
    out_d = nc.dram_tensor("out", [TS + P, D], f32, kind="ExternalOutput")

    with tile.TileContext(nc) as tc:
        with (
            tc.tile_pool(name="const", bufs=1) as cpool,
            tc.tile_pool(name="xt", bufs=2) as xpool,
            tc.tile_pool(name="xt_t", bufs=1) as xpool_t,
            tc.tile_pool(name="osb", bufs=2) as opool,
            tc.tile_pool(name="osb_t", bufs=1) as opool_t,
            tc.tile_pool(name="ps", bufs=4, space="PSUM") as pspool,
            tc.tile_pool(name="ps_b", bufs=1, space="PSUM") as ps_b,
        ):
            nc.gpsimd.load_library(mlp)
            gidx_sb = cpool.tile([128, S * P // 16], i16, tag="gidx")
            nc.sync.dma_start(gidx_sb[:], gi_d[:, :])
            sidx_sb = cpool.tile([128, S * P // 16], i16, tag="sidx")
            nc.sync.dma_start(sidx_sb[:], si_d[:, :])

            # resident weights: one [P, D] bf16 SBUF tile per (branch, k)
            w_sb = {}
            for n in range(NB):
                for k in range(KC):
                    w = cpool.tile([P, D], bf16, tag=f"w_{n}_{k}")
                    r0 = (n * KC + k) * P
                    nc.sync.dma_start(w[:], w_d[r0:r0 + P, :])
                    w_sb[(n, k)] = w

            # bias, broadcast to 128 partitions via K=1 matmul with ones
            bias1p = cpool.tile([1, NB * D], f32, tag="bias1p")
            nc.sync.dma_start(bias1p[:], b_d[:, :])
            ones1p = cpool.tile([1, P], f32, tag="ones1p")
            nc.vector.memset(ones1p[:], 1.0)
            bias_bc = cpool.tile([P, NB * D], f32, tag="bias_bc")
            for n in range(NB):
                pb = ps_b.tile([P, D], f32)
                nc.tensor.matmul(
                    out=pb[:], lhsT=ones1p[:], rhs=bias1p[:, n * D:(n + 1) * D],
                    start=True, stop=True,
                )
                nc.scalar.copy(out=bias_bc[:, n * D:(n + 1) * D], in_=pb[:])

            chunks = [(c0, min(c0 + TPC, S)) for c0 in range(0, S, TPC)]
            # one shared Pool register per distinct num_idxs (a fresh to_reg
            # per call exhausts the engine register file at epochs=8)
            nidx_regs = {}
            for (c0, c1) in chunks:
                nidx = (c1 - c0) * P
                if nidx not in nidx_regs:
                    nidx_regs[nidx] = nc.gpsimd.to_reg(nidx)
            def emit_gather(c0, c1):
                # gather+transpose sorted bf16 rows:
                #   xt[p, k, i] = x16[gidx[i], k*128 + p]
                nt = c1 - c0
                nidx = nt * P
                xp = xpool if nt == TPC else xpool_t
                xt = xp.tile([P, KC, nidx], bf16, tag=f"xt{nt}")
                nc.gpsimd.dma_gather(
                    xt[:], x_d[:, :],
                    gidx_sb[:, c0 * P // 16:c1 * P // 16],
                    nidx, nidx_regs[nidx], D, transpose=True,
                    queue_num=0, single_packet=False)
                return xt

            def emit_compute_scatter(c0, c1, xt):
                nt = c1 - c0
                nidx = nt * P
                op = opool if nt == TPC else opool_t
                osb = op.tile([P, nt, D], f32, tag=f"osb{nt}")
                for j in range(nt):
                    n = branch_of_tile[c0 + j]
                    # out[tok,:] = sum_k xt[:,k,tile].T @ W[n][k]
                    ps = pspool.tile([P, D], f32)
                    for k in range(KC):
                        nc.tensor.matmul(
                            out=ps[:],
                            lhsT=xt[:, k, j * P:(j + 1) * P],
                            rhs=w_sb[(n, k)][:],
                            start=(k == 0), stop=(k == KC - 1),
                        )
                    # + bias (PSUM -> SBUF)
                    nc.vector.tensor_add(
                        out=osb[:, j, :], in0=ps[:],
                        in1=bias_bc[:, n * D:(n + 1) * D],
                    )
                # scatter rows to their original token slots
                nc.gpsimd.dma_scatter_add(
                    out_d[:, :], osb[:],
                    sidx_sb[:, c0 * P // 16:c1 * P // 16],
                    nidx, nidx_regs[nidx], D, queue_num=0,
                    single_packet=False)

            # software pipeline: issue gather c+1 before scatter c, so the
            # Pool engine's gather stream never stalls on chunk-c compute
            seq = chunks * epochs
            xt_prev = emit_gather(*seq[0])
            for i, (c0, c1) in enumerate(seq):
                xt_next = emit_gather(*seq[i + 1]) if i + 1 < len(seq) else None
                emit_compute_scatter(c0, c1, xt_prev)
                xt_prev = xt_next
    _split_multiwaits(nc)
    # raw Bass skips Bacc's codegen pass that fills .instr for extended-inst
    # InstISA subclasses (load_library); without it walrus sees empty bytes
    # ("ISA wrong length")
    mybir.codegen_inst_isa_subclasses(nc)
    return nc


def _routing(branch_idx, TS, NB):
    """Per-core branch-sorted gather/scatter index arrays (Ant layout).

    Returns (gidx [NCORES][128, S*8] i16, sidx likewise, branch_of_tile).
    Linear slot n maps to [n % 16, n // 16], replicated across the 8 Q7
    core windows. Gather pads read row 0; scatter pads add into trash
    rows TS + (slot % 128)."""
    ncores = branch_idx.shape[0] // TS
    perms, counts = [], np.zeros((ncores, NB), np.int64)
    for c in range(ncores):
        bi = branch_idx[c * TS:(c + 1) * TS]
        perms.append(np.argsort(bi, kind="stable"))
        counts[c] = np.bincount(bi, minlength=NB)
    slot_tiles = [int(-(-counts[:, n].max() // P)) for n in range(NB)]
    branch_of_tile = []
    for n in range(NB):
        branch_of_tile += [n] * slot_tiles[n]
    S = len(branch_of_tile)

    def wrap(flat):
        # linear slot n at [n % 16, n // 16]; 16-row pattern replicated to
        # all 8 Q7 core windows (each queue's core pair reads its own one)
        return np.ascontiguousarray(np.tile(flat.reshape(-1, 16).T, (8, 1)))

    gidx_arrays, sidx_arrays = [], []
    for c in range(ncores):
        flat = np.full(S * P, -1, np.int64)
        off = base = 0
        for n in range(NB):
            cnt = int(counts[c, n])
            flat[base:base + cnt] = perms[c][off:off + cnt]
            off += cnt
            base += slot_tiles[n] * P
        pad = flat < 0
        gflat = np.where(pad, 0, flat).astype(np.int16)
        sflat = np.where(pad, TS + (np.arange(S * P) % P), flat).astype(np.int16)
        gidx_arrays.append(wrap(gflat))
        sidx_arrays.append(wrap(sflat))
    return gidx_arrays, sidx_arrays, branch_of_tile


def kernel(x, branch_idx, weight, bias):
    from concourse.bass_utils import run_bass_kernel_spmd

    x = np.asarray(x, np.float32)
    branch_idx = np.asarray(branch_idx, np.int32)
    weight = np.asarray(weight, np.float32)
    bias = np.asarray(bias, np.float32)

    T, D = x.shape
    NB = weight.shape[0]
    TS = T // NCORES

    gidx_arrays, sidx_arrays, branch_of_tile = _routing(branch_idx, TS, NB)

    key = (TS, D, NB, tuple(branch_of_tile))
    if key not in _prog_cache:
        _prog_cache[key] = _build_program(TS, D, NB, branch_of_tile)
    nc = _prog_cache[key]

    x16 = np.ascontiguousarray(x.astype(ml_dtypes.bfloat16))
    w16 = np.ascontiguousarray(
        weight.reshape(NB * D, D).astype(ml_dtypes.bfloat16))
    br = np.ascontiguousarray(bias.reshape(1, NB * D))
    in_maps = [
        {"x16": x16[c * TS:(c + 1) * TS], "w16": w16, "br": br,
         "gidx": gidx_arrays[c], "sidx": sidx_arrays[c]}
        for c in range(NCORES)
    ]
    res = run_bass_kernel_spmd(nc, in_maps, core_ids=list(range(NCORES)))
    out = np.concatenate(
        [res.results[c]["out"][:TS] for c in range(NCORES)], axis=0)
    return out
